# revision 1
# baseline (speedup 1.0000x reference)
"""
MoE-routing kernel for Trainium2 (8 NeuronCores, SPMD via bass).

Computation (matches the reference):
  attended[b, c] = sum_hw(mn[b, hw] * feat[b, c, hw]),  mn = (m+1e-10)/sum(m+1e-10)
  out[b, a]      = attended[b, :] @ W[inst[b], a, :] + bias[inst[b], a]

Strategy: channel-sharded over 8 cores (CS = 2048/8 = 256 channels each);
host sums the 8 partial [B, A] outputs and adds the bias.  Samples are
sorted by expert on the host so each expert's samples form contiguous
stationary columns.  All streamed tensors (feat, mask, W) are cast to
fp16 on the host, halving DMA traffic; accumulations stay fp32 on device.

Per core:
  phase 1 (pooling on the PE): feat is host-transposed to [s, hw, c] and
    DMA'd 4 samples at a time with hw on partitions (2 k-tiles of 98).
    For each sample and 128-channel tile, matmul(stationary=feat^T[hw,c],
    moving=mn[s] column) contracts hw -- the mask multiply rides inside
    the matmul, so no DVE work and the result lands directly in att^T
    [c, s] layout (psum columns, evicted in 64-sample blocks as fp16).
  phase 2 (grouped GEMM): per expert, one whole-weight DMA [128, KT, A]
    (6 KB descriptors); per group and 512-answer chunk, two fp16 matmuls
    accumulate in PSUM; Activation engine evicts to an SBUF row tile
    which DMAs out once per group (6 KB descriptors).
DMA queues: feat+mask+out on SP HWDGE, weights on Activation HWDGE.
"""

import sys

if "/opt/trn_rl_repo" not in sys.path:
    sys.path.insert(0, "/opt/trn_rl_repo")

import numpy as np

import concourse.bass as bass
import concourse.mybir as mybir
import concourse.tile as tile
from concourse import bacc
from concourse import bass_utils

# Problem constants (hardcoded; kernel.py must be self-contained)
B = 256          # batch
C = 2048         # channels
HW = 196         # spatial positions (14*14)
E = 16           # experts
A = 3000         # answers
NCORES = 8
CS = C // NCORES  # channel shard per core = 256
P = 128
KT = CS // P      # c k-tiles per core = 2
HWT = 2           # hw k-tiles
HWP = HW // HWT   # hw partitions per k-tile = 98
SB = 4            # samples per feat DMA
BLK = 64          # samples per psum evict block
CHUNKS = [(c0, min(512, A - c0)) for c0 in range(0, A, 512)]

F32 = mybir.dt.float32
F16 = mybir.dt.float16


def _make_groups(counts):
    """[(gstart_in_sorted_order, gsz, expert)] with gsz <= 128."""
    groups = []
    start = 0
    for e in range(E):
        n = int(counts[e])
        g0 = start
        while n > 0:
            gsz = min(n, P)
            groups.append((g0, gsz, e))
            g0 += gsz
            n -= gsz
        start += int(counts[e])
    return groups


def build_program(groups, loop_n=1, do_pool=True, do_mm=True):
    """Build + compile the per-core Bass program (identical on all cores)."""
    nc = bacc.Bacc("TRN2", target_bir_lowering=False, debug=False,
                   num_devices=NCORES)

    feat_d = nc.dram_tensor("feat", [B // SB, HWP, SB, HWT, CS], F16,
                            kind="ExternalInput").ap()
    mask_d = nc.dram_tensor("mask", [HWP, HWT, B], F16, kind="ExternalInput").ap()
    wt_d = nc.dram_tensor("wt", [E, KT, P, A], F16, kind="ExternalInput").ap()
    part_d = nc.dram_tensor("part", [B, A], F16, kind="ExternalOutput").ap()

    import contextlib
    with tile.TileContext(nc) as tc:
        loop_ctx = tc.For_i(0, loop_n, 1) if loop_n > 1 else contextlib.nullcontext()
        with (
            loop_ctx,
            tc.tile_pool(name="persist", bufs=1) as pp,
            tc.tile_pool(name="feat", bufs=6) as fp,
            tc.tile_pool(name="wt", bufs=3) as wtp,
            tc.tile_pool(name="outs", bufs=2) as op,
            tc.tile_pool(name="ps_mm", bufs=3, space="PSUM") as pmm,
            tc.tile_pool(name="ps_pool", bufs=2, space="PSUM") as psp,
        ):
            mk = pp.tile([HWP, HWT, B], F16, tag="mask")
            nc.sync.dma_start(mk, mask_d)

            att_T = pp.tile([P, KT, B], F16, tag="attT")
            if not do_pool:
                nc.vector.memset(att_T.bitcast(F32), 0.0)

            # ---- phase 1: pooling on the PE ----
            if do_pool:
                for blk in range(B // BLK):
                    pss = psp.tile([P, KT, BLK], F32, name="psp")
                    for j in range(BLK // SB):
                        sb = blk * (BLK // SB) + j
                        ft = fp.tile([HWP, SB, HWT, CS], F16, tag="feat")
                        nc.sync.dma_start(ft, feat_d[sb])
                        for i in range(SB):
                            s = sb * SB + i
                            pos = j * SB + i
                            for ct in range(KT):
                                for t in range(HWT):
                                    nc.tensor.matmul(
                                        pss[:, ct, pos:pos + 1],
                                        lhsT=ft[:, i, t, ct * P:(ct + 1) * P],
                                        rhs=mk[:, t, s:s + 1],
                                        start=(t == 0), stop=(t == HWT - 1))
                    nc.vector.tensor_copy(
                        att_T[:, :, blk * BLK:(blk + 1) * BLK], pss)

            # ---- phase 2: grouped GEMM, whole-expert weight loads ----
            for gi, (g0, gsz, e) in enumerate(groups):
                wt = wtp.tile([P, KT, A], F16, tag="wt")
                nc.scalar.dma_start(wt, wt_d[e].rearrange("t p a -> p t a"))
                ot = op.tile([P, A], F16, tag="out")
                for (c0, cw) in CHUNKS:
                    if not do_mm:
                        continue
                    ps = pmm.tile([P, 512], F32, name="ps")
                    for t in range(KT):
                        nc.tensor.matmul(
                            ps[:gsz, :cw],
                            lhsT=att_T[:, t, g0:g0 + gsz],
                            rhs=wt[:, t, c0:c0 + cw],
                            start=(t == 0), stop=(t == KT - 1))
                    nc.scalar.copy(ot[:gsz, c0:c0 + cw], ps[:gsz, :cw])
                if not do_mm:
                    nc.vector.memset(ot[:gsz].bitcast(F32), 0.0)
                nc.sync.dma_start(part_d[g0:g0 + gsz, :], ot[:gsz])

    nc.compile()
    return nc


_PROGRAM_CACHE = {}


def _get_program(groups):
    key = tuple(groups)
    if key not in _PROGRAM_CACHE:
        _PROGRAM_CACHE[key] = build_program(groups)
    return _PROGRAM_CACHE[key]


def make_in_maps(mask, features, W, b, inst):
    """Host-side routing + sharding.  Returns (in_maps, perm, groups)."""
    inst_np = np.asarray(inst).astype(np.int64)
    perm = np.argsort(inst_np, kind="stable")
    counts = np.bincount(inst_np, minlength=E)
    groups = _make_groups(counts)

    m = np.asarray(mask, np.float64).reshape(B, HW) + 1e-10
    mn = (m / m.sum(1, keepdims=True)).astype(np.float16)[perm]
    # mask_h[p, t, s] = mn[s, t*HWP + p]
    mask_h = np.ascontiguousarray(mn.reshape(B, HWT, HWP).transpose(2, 1, 0))

    feat = np.asarray(features, np.float32).reshape(B, C, HW)[perm]
    Wf = np.asarray(W, np.float32)

    in_maps = []
    for k in range(NCORES):
        sl = slice(k * CS, (k + 1) * CS)
        # feat_k[sb, p, i, t, c] = feat[sb*SB+i, c_k, t*HWP + p]
        feat_k = np.ascontiguousarray(
            feat[:, sl].reshape(B // SB, SB, CS, HWT, HWP)
            .transpose(0, 4, 1, 3, 2)).astype(np.float16)
        # wt_k[e, t, p, a] = W[e, a, k*CS + t*128 + p]
        wt_k = np.ascontiguousarray(
            Wf[:, :, sl].transpose(0, 2, 1).reshape(E, KT, P, A)).astype(np.float16)
        in_maps.append({
            "feat": feat_k,
            "mask": mask_h,
            "wt": wt_k,
        })
    return in_maps, perm, groups


def postprocess(results, perm, b, inst):
    part = np.zeros((B, A), np.float32)
    for r in results:
        part += np.asarray(r["part"], np.float32)
    out = np.empty((B, A), np.float32)
    out[perm] = part
    out += np.asarray(b, np.float32)[np.asarray(inst).astype(np.int64)]
    return out


def kernel(mask, features, W, b, inst):
    in_maps, perm, groups = make_in_maps(mask, features, W, b, inst)
    nc = _get_program(groups)
    res = bass_utils.run_bass_kernel_spmd(nc, in_maps, core_ids=list(range(NCORES)))
    return postprocess(res.results, perm, b, inst)

